# revision 50
# baseline (speedup 1.0000x reference)
"""Trainium2 Bass kernel for nn_Attention_35021163332119.

Full multi-head attention: qkv = x @ w_qkv; RoPE(q, k); softmax(q k^T / sqrt(dh)) v;
out = heads @ w_out + b_out.  B=2, N=2048, DIM=1024, H=16, DH=64.

Sharding: 8 cores = (batch b in {0,1}) x (head-group g in {0..3} of 4 heads).
Each core computes its 4 heads end-to-end plus the partial output projection
for its head-group's rows of w_out; the host sums the 4 partials per batch
and adds b_out.

The schedule is built around the Activation engine: softmax exp is
Act-only (1 elem/cycle/partition at 1.2GHz -> ~129us for the 16.8M
scores/core) and nothing else can run it, so the kernel starts the exp
stream as early as possible and hides ALL other PE work inside the
Act-paced attention window (~262us baseline -> ~228us):
- Inputs DMA in consumption order (xT/w k-tiles first, split halves);
  pair-0 QKV runs k-OUTER so matmuls start when the first xT k-tile
  lands; RoPE is applied per 512-column chunk behind each pass.
- attention(0,0) begins once pair-0 RoPE and v0-7 are done (~45us, the
  phase-1 critical path); v8-15, ALL pair-1 QKV (as k-quarter jobs with
  an SBUF staging add) + its RoPE, and the output projection run as
  "filler" jobs woven one-per-j-iteration into the PE slack of later
  blocks.  Fillers never defer work a block reads (k columns are needed
  by every block's j-loop; proj(iq) lags >= 2 blocks past normalize).
- Scalar engine: exp only during the window (phase-1 v copies ride on it
  while idle); all other PSUM->SBUF copies on DVE.
- Scores matmul pairs carry tile_position (0,0)/(64,0) — the K=64 pair
  co-issues into disjoint PE row groups when PSUM frees line up.
- q/k tiles are bf16 (halves SBUF, 64-row weight loads); the partial
  output is written bf16 (halves the output DMA).  rel err ~1.1e-2 vs
  the 2e-2 gate (bf16 q/k contributes ~5e-3 over the f32r variant).
- PSUM: 3x[128,1024] score/filler bufs + 2x[65,512] PV accumulators = 8
  banks.  Fewer score bufs puts semaphore latency on the exp critical
  path (+40us); separate filler banks starve the score pipeline.
"""

import numpy as np

B, N, DIM, H, DH = 2, 2048, 1024, 16, 64
ROPE_BASE = 10000.0
SCALE = DH ** -0.5
N_CORES = 8
G = 4                 # heads per core
KT = DIM // 128       # contraction tiles
NT = N // 128         # sequence tiles

_cache = {}


def _rope_tables():
    inv_freq = (1.0 / (ROPE_BASE ** (np.arange(0, DH, 2, dtype=np.float32) / DH)))
    t = np.arange(N, dtype=np.float32)
    freqs = t[:, None] * inv_freq[None, :]          # [N, DH/2]
    freqs = np.repeat(freqs, 2, axis=-1)            # [N, DH] interleaved
    cosT = np.cos(freqs).T.astype(np.float32)       # [DH, N]
    sinT = np.sin(freqs).T.astype(np.float32)
    cos2 = np.concatenate([cosT, cosT], axis=0)     # [128, N] two heads stacked
    sin2 = np.concatenate([sinT, sinT], axis=0)
    return np.ascontiguousarray(cos2), np.ascontiguousarray(sin2)


def _p2t():
    # rot = P2 @ qT with P2 = blockdiag(P, P), P[2t, 2t+1] = -1, P[2t+1, 2t] = 1
    # matmul computes lhsT.T @ rhs, so pass P2.T
    p = np.zeros((DH, DH), dtype=np.float32)
    for t in range(DH // 2):
        p[2 * t, 2 * t + 1] = -1.0
        p[2 * t + 1, 2 * t] = 1.0
    p2 = np.zeros((128, 128), dtype=np.float32)
    p2[:DH, :DH] = p
    p2[DH:, DH:] = p
    return np.ascontiguousarray(p2.T)


def _build():
    if "nc" in _cache:
        return _cache["nc"]

    import concourse.mybir as mybir
    import concourse.tile as tile
    from concourse import bacc

    F32 = mybir.dt.float32
    BF16 = mybir.dt.bfloat16
    EXP = mybir.ActivationFunctionType.Exp

    nc = bacc.Bacc("TRN2", target_bir_lowering=False, debug=False)
    xT_d = nc.dram_tensor("xT", [DIM, N], BF16, kind="ExternalInput")
    w_d = nc.dram_tensor("wqkv", [DIM, 768], BF16, kind="ExternalInput")
    wout_d = nc.dram_tensor("wout", [G * DH, DIM], BF16, kind="ExternalInput")
    cos_d = nc.dram_tensor("cos2", [128, N], BF16, kind="ExternalInput")
    sin_d = nc.dram_tensor("sin2", [128, N], BF16, kind="ExternalInput")
    p2t_d = nc.dram_tensor("p2t", [128, 128], BF16, kind="ExternalInput")
    part_d = nc.dram_tensor("part", [N, DIM], BF16, kind="ExternalOutput")

    with tile.TileContext(nc) as tc:
        with tc.tile_pool(name="persist", bufs=1) as persist, \
             tc.tile_pool(name="att", bufs=5) as att, \
             tc.tile_pool(name="norm_w", bufs=2) as norm_w, \
             tc.tile_pool(name="outp", bufs=3) as outp, \
             tc.tile_pool(name="xph", bufs=1) as xph, \
             tc.tile_pool(name="rope_w", bufs=2) as rope_w, \
             tc.tile_pool(name="ps", bufs=3, space="PSUM") as ps, \
             tc.tile_pool(name="pso", bufs=2, space="PSUM") as pso:

            # ---- persistent tiles ----
            qk_sb = [persist.tile([128, N], BF16, tag=f"qk{m}", name=f"qk{m}")
                     for m in range(4)]          # q01T, q23T, k01T, k23T
            v_aug = persist.tile([128, NT, G, DH + 1], BF16, tag="vaug")
            wout_sb = [persist.tile([128, DIM], BF16, tag=f"wo{kk}", name=f"wo{kk}")
                       for kk in range(2)]
            outT = [persist.tile([128, N], BF16, tag=f"outT{p}", name=f"outT{p}")
                    for p in range(2)]

            # ---- phase-1 tiles ----
            xT = [xph.tile([128, N], BF16, tag=f"xT{k}", name=f"xT{k}")
                  for k in range(KT)]
            wqkv = [xph.tile([128, 768], BF16, tag=f"wq{k}", name=f"wq{k}")
                    for k in range(KT)]
            cos2 = xph.tile([128, N], BF16, tag="cos2")
            sin2 = xph.tile([128, N], BF16, tag="sin2")
            p2t = xph.tile([128, 128], BF16, tag="p2t")
            ones_col = xph.tile([128, NT, G, 1], F32, tag="ones")

            # ---- input DMA in consumption order ----
            xT_r = xT_d.ap().rearrange("(t p) n -> t p n", p=128)
            w_r = w_d.ap().rearrange("(t p) m -> t p m", p=128)
            for k in range(KT):
                nc.sync.dma_start(out=xT[k][:, 0:1024], in_=xT_r[k][:, 0:1024])
                nc.sync.dma_start(out=wqkv[k], in_=w_r[k])
            nc.sync.dma_start(out=cos2[:, 0:1024], in_=cos_d.ap()[:, 0:1024])
            nc.sync.dma_start(out=sin2[:, 0:1024], in_=sin_d.ap()[:, 0:1024])
            nc.sync.dma_start(out=p2t, in_=p2t_d.ap())
            for k in range(KT):
                nc.sync.dma_start(out=xT[k][:, 1024:2048],
                                  in_=xT_r[k][:, 1024:2048])
            nc.sync.dma_start(out=cos2[:, 1024:2048],
                              in_=cos_d.ap()[:, 1024:2048])
            nc.sync.dma_start(out=sin2[:, 1024:2048],
                              in_=sin_d.ap()[:, 1024:2048])
            for kk in range(2):
                nc.sync.dma_start(
                    out=wout_sb[kk],
                    in_=wout_d.ap().rearrange("(t p) m -> t p m", p=128)[kk])
            nc.vector.memset(ones_col, 1.0)

            def qk_pass0(cpass, ms=(0, 2)):
                """k-outer accumulation of pair-0 q/k tiles for one 1024-wide
                column chunk; consumes each xT k-tile as its DMA lands."""
                csl = slice(cpass * 1024, (cpass + 1) * 1024)
                psq = [ps.tile([128, 1024], F32, tag="s", name=f"qk_ps{m2}")
                       for m2 in range(len(ms))]
                for k in range(KT):
                    for m2, m in enumerate(ms):
                        for half in range(2):
                            hsl = slice(half * 512, (half + 1) * 512)
                            xsl = slice(cpass * 1024 + half * 512,
                                        cpass * 1024 + (half + 1) * 512)
                            nc.tensor.matmul(
                                psq[m2][:, hsl],
                                wqkv[k][:, m * 128:(m + 1) * 128],
                                xT[k][:, xsl],
                                start=(k == 0), stop=(k == KT - 1))
                for m2, m in enumerate(ms):
                    nc.vector.tensor_copy(qk_sb[m][:, csl], psq[m2])

            def rope_chunk(m, c):
                """RoPE on one 512-col chunk of qk_sb[m]: rotate-half via the
                PE permutation matmul, then combine with cos/sin on DVE."""
                csl = slice(c * 512, (c + 1) * 512)
                rp = ps.tile([128, 1024], F32, tag="s", name="rot")
                rps = rp[:, 0:512]
                nc.tensor.matmul(rps, p2t, qk_sb[m][:, csl],
                                 start=True, stop=True)
                tmp = rope_w.tile([128, 512], BF16, tag="rtmp")
                nc.vector.tensor_mul(tmp, rps, sin2[:, csl])
                nc.vector.tensor_mul(qk_sb[m][:, csl], qk_sb[m][:, csl],
                                     cos2[:, csl])
                nc.vector.tensor_add(qk_sb[m][:, csl], qk_sb[m][:, csl], tmp)

            def v_group(tn, cp_act=False):
                mm_ps = ps.tile([128, 1024], F32, tag="s", name="mm_v")
                for k in range(KT):
                    nc.tensor.matmul(
                        mm_ps[:, 0:G * DH],
                        xT[k][:, tn * 128:(tn + 1) * 128],
                        wqkv[k][:, 512:768],
                        start=(k == 0), stop=(k == KT - 1))
                src = mm_ps[:, 0:G * DH].rearrange("p (h d) -> p h d", h=G)
                if cp_act:
                    nc.scalar.copy(v_aug[:, tn, :, 0:DH], src)
                else:
                    nc.vector.tensor_copy(v_aug[:, tn, :, 0:DH], src)
                nc.vector.tensor_copy(v_aug[:, tn, :, DH:DH + 1],
                                      ones_col[:, tn])

            # pair-1 QKV as filler jobs: k-QUARTER per (m2, 512-col chunk) so
            # each job holds a PSUM buf only ~1us; halves are combined via an
            # SBUF staging tile and a DVE add.
            p1_stage = {}

            def pass1_q(m2, chunk, q):
                m = 1 if m2 == 0 else 3
                csl = slice(chunk * 512, (chunk + 1) * 512)
                tt = ps.tile([128, 1024], F32, tag="s", name="p1")
                t = tt[:, 0:512]
                for k in range(q * 4, q * 4 + 4):
                    nc.tensor.matmul(
                        t,
                        wqkv[k][:, m * 128:(m + 1) * 128], xT[k][:, csl],
                        start=(k == q * 4), stop=(k == q * 4 + 3))
                if q == 0:
                    st = rope_w.tile([128, 512], F32, tag="p1st")
                    p1_stage[(m2, chunk)] = st
                    nc.vector.tensor_copy(st, t)
                else:
                    st = p1_stage.pop((m2, chunk))
                    nc.vector.tensor_add(qk_sb[m][:, csl], st, t)

            def normalize(p, iq, o_ps):
                """PV accumulators -> normalized bf16 rows of outT.  The two
                head chains are interleaved stage-by-stage so their DMA and
                gpsimd latencies overlap instead of serializing."""
                isl = slice(iq * 512, iq * 512 + 512)
                o_sb = []
                recips = []
                bcs = []
                for hh in range(2):
                    t = norm_w.tile([DH + 1, 512], F32, tag=f"osb{hh}",
                                    name=f"osb{hh}")
                    nc.vector.tensor_copy(t, o_ps[hh])
                    o_sb.append(t)
                for hh in range(2):
                    r = norm_w.tile([1, 512], F32, tag=f"r0{hh}",
                                    name=f"r0{hh}")
                    nc.sync.dma_start(out=r, in_=o_sb[hh][DH:DH + 1, :])
                    recips.append(r)
                for hh in range(2):
                    nc.vector.reciprocal_approx_fast(recips[hh], recips[hh])
                for hh in range(2):
                    bc = norm_w.tile([DH, 512], F32, tag=f"bc{hh}",
                                     name=f"bc{hh}")
                    nc.gpsimd.partition_broadcast(bc, recips[hh])
                    bcs.append(bc)
                nc.vector.tensor_mul(outT[p][0:DH, isl],
                                     o_sb[0][0:DH, :], bcs[0])
                tmpb = norm_w.tile([DH, 512], BF16, tag="tmpb")
                nc.vector.tensor_mul(tmpb, o_sb[1][0:DH, :], bcs[1])
                nc.sync.dma_start(out=outT[p][DH:2 * DH, isl], in_=tmpb)

            def emit_pv(p, o_ps, jj, exps):
                for hh in range(2):
                    for half in range(2):
                        j = 2 * jj + half
                        nc.tensor.matmul(
                            o_ps[hh],
                            v_aug[:, j, 2 * p + hh, :],
                            exps[hh][:, half * 512:(half + 1) * 512],
                            start=(j == 0), stop=(j == NT - 1))

            def run_attention(blocks):
                """All attention blocks as ONE continuous scores -> exp -> PV
                pipeline.  PV lags exp by one iteration ACROSS block
                boundaries, so each next block's first scores sit between a
                block's last exp and its last PV in PE program order — the
                exp stream never waits out a block tail.  Filler jobs pop
                into PE slack at the given j-slots."""
                pend = None      # (p, iq, o_ps, jj, exps)
                for p, iq, fillers, slots in blocks:
                    o_ps = [pso.tile([DH + 1, 512], F32, tag="o",
                                     name=f"o{hh}") for hh in range(2)]
                    qT = qk_sb[p]
                    kTt = qk_sb[2 + p]
                    isl = slice(iq * 512, iq * 512 + 512)
                    fillers = list(fillers)
                    if slots is None:
                        slots = list(range(len(fillers)))
                    fi = 0
                    for jj in range(NT // 2):
                        s_ps = [ps.tile([128, 1024], F32, tag="s",
                                        name=f"s{hh}") for hh in range(2)]
                        for half in range(2):
                            j = 2 * jj + half
                            jsl = slice(j * 128, (j + 1) * 128)
                            for hh in range(2):
                                hsl = slice(hh * DH, (hh + 1) * DH)
                                # K=64 pair in disjoint PE row groups can
                                # co-issue (explicit tile_position)
                                nc.tensor.matmul(
                                    s_ps[hh][:, half * 512:(half + 1) * 512],
                                    kTt[hsl, jsl], qT[hsl, isl],
                                    start=True, stop=True,
                                    tile_position=(hh * DH, 0))
                        exps = []
                        for hh in range(2):
                            expT = att.tile([128, 1024], BF16, tag="exp")
                            nc.scalar.activation(expT, s_ps[hh], EXP,
                                                 scale=SCALE)
                            exps.append(expT)
                        if pend is not None:
                            pp, piq, po, pjj, pexps = pend
                            emit_pv(pp, po, pjj, pexps)
                            if pjj == NT // 2 - 1:
                                normalize(pp, piq, po)
                        pend = (p, iq, o_ps, jj, exps)
                        while fi < len(fillers) and fi < len(slots) \
                                and slots[fi] <= jj:
                            fillers[fi]()
                            fi += 1
                    while fi < len(fillers):
                        fillers[fi]()
                        fi += 1
                pp, piq, po, pjj, pexps = pend
                emit_pv(pp, po, pjj, pexps)
                normalize(pp, piq, po)

            def proj_tile(tn, cp_act=False):
                nsl = slice(tn * 128, (tn + 1) * 128)
                out_sb = outp.tile([128, DIM], BF16, tag="osb")
                f_ps = ps.tile([128, 1024], F32, tag="s", name="f_ps")
                for c2 in range(2):
                    c2sl = slice(c2 * 512, (c2 + 1) * 512)
                    for kk in range(2):
                        nc.tensor.matmul(
                            f_ps[:, c2sl],
                            outT[kk][:, nsl], wout_sb[kk][:, c2sl],
                            start=(kk == 0), stop=(kk == 1))
                if cp_act:
                    nc.scalar.copy(out_sb, f_ps)
                else:
                    nc.vector.tensor_copy(out_sb, f_ps)
                nc.sync.dma_start(
                    out=part_d.ap().rearrange("(t p) m -> t p m", p=128)[tn],
                    in_=out_sb)

            # ---- emission order ----
            # Inline phase 1 is the minimum needed by attention(0,0):
            # pair-0 qk for columns 0:1024 (k-outer, consumes xT as the DMA
            # lands), its rope, and v tiles 0-7 (copies on the idle scalar
            # engine).  Everything else — pair-0 columns 1024:2048, ALL of
            # pair-1 qkv+rope, v8-15, and the projection — is filler jobs
            # inside the Act-paced attention window.
            qk_pass0(0)
            for m in (0, 2):
                for c in (0, 1):
                    rope_chunk(m, c)
            for tn in range(0, 8):
                v_group(tn, cp_act=True)
            qk_pass0(1)
            for m in (0, 2):
                for c in (2, 3):
                    rope_chunk(m, c)

            def F(fn, *a):
                return lambda: fn(*a)

            # Deadlines: attention(p, iq) scores at jj=4/6 read k-chunks
            # 2/3 (filler+rope one block earlier or early-slot same block);
            # attention(1,*) reads roped qk_sb[1]/[3] produced in the
            # (0,2)/(0,3) blocks.  proj(iq) lags >= 2 blocks.
            f_v = [F(v_group, tn) for tn in range(8, 16)]
            f_p1 = [F(pass1_q, m2, c, q)
                    for m2 in range(2) for c in range(4) for q in range(2)]
            # pair-1 rope split: only q23-c0/c1 and k23-c0/c1 are read in the
            # first half of att(1,0)'s j-loop; the later chunks can rope as
            # EARLY-slot fillers inside att(1,0) itself (the write is emitted
            # before the jj>=4 scores that read it), spreading filler load.
            f_r1a = [F(rope_chunk, 1, 0), F(rope_chunk, 3, 0),
                     F(rope_chunk, 3, 1), F(rope_chunk, 1, 1)]
            f_r1b = [F(rope_chunk, 3, 2), F(rope_chunk, 3, 3),
                     F(rope_chunk, 1, 2), F(rope_chunk, 1, 3)]

            run_attention([
                # v fillers start at slot 1: a slot-0 filler lands exactly in
                # the score-pipeline fill of the first block (v15 runs as the
                # post-loop leftover, still ahead of the cross-block PV(7))
                (0, 0, f_v, [1, 2, 3, 4, 5, 6, 7]),
                (0, 1, f_p1[0:8], None),
                (0, 2, f_p1[8:16], None),
                (0, 3, f_r1a, [1, 3, 5, 7]),
                (1, 0, f_r1b, [0, 1, 2, 4]),
                (1, 1, [F(proj_tile, tn) for tn in range(0, 4)],
                 [4, 5, 6, 7]),
                (1, 2, [F(proj_tile, tn) for tn in range(4, 8)],
                 [4, 5, 6, 7]),
                (1, 3, [F(proj_tile, tn) for tn in range(8, 10)], [5, 7]),
            ])
            # proj 10,11 depend only on normalize(1,2): they keep the PE warm
            # while normalize(1,3) drains; 12-15 follow it, staged into one
            # SBUF tile and written with a single DMA (one issue+sem instead
            # of four on the critical tail).
            for tn in range(10, 12):
                proj_tile(tn)
            tail_sb = persist.tile([128, 4, DIM], BF16, tag="tail")
            for ti, tn in enumerate(range(12, 16)):
                nsl = slice(tn * 128, (tn + 1) * 128)
                f_ps = ps.tile([128, 1024], F32, tag="s", name="f_ps")
                for c2 in range(2):
                    c2sl = slice(c2 * 512, (c2 + 1) * 512)
                    for kk in range(2):
                        nc.tensor.matmul(
                            f_ps[:, c2sl],
                            outT[kk][:, nsl], wout_sb[kk][:, c2sl],
                            start=(kk == 0), stop=(kk == 1))
                if tn % 2 == 0:
                    nc.scalar.copy(tail_sb[:, ti], f_ps)
                else:
                    nc.vector.tensor_copy(tail_sb[:, ti], f_ps)
            nc.sync.dma_start(
                out=part_d.ap().rearrange("(t p) m -> p t m", p=128)[:, 12:16],
                in_=tail_sb)
    nc.compile()
    _cache["nc"] = nc
    return nc


def kernel(x, w_qkv, w_out, b_out, _trace=False):
    import ml_dtypes
    from concourse.bass_utils import run_bass_kernel_spmd

    x = np.asarray(x, dtype=np.float32)
    w_qkv = np.asarray(w_qkv, dtype=np.float32)
    w_out = np.asarray(w_out, dtype=np.float32)
    b_out = np.asarray(b_out, dtype=np.float32)

    cos2, sin2 = _rope_tables()
    p2t = _p2t()

    in_maps = []
    for c in range(N_CORES):
        b, g = divmod(c, G)
        cols = []
        for blk in range(2):                      # q block, k block
            base = blk * H * DH + g * G * DH
            cols.append(w_qkv[:, base:base + G * DH])
        cols.append(w_qkv[:, 2 * H * DH + g * G * DH:
                          2 * H * DH + (g + 1) * G * DH])   # v block
        wqkv_c = np.ascontiguousarray(np.concatenate(cols, axis=1))  # [DIM,768]
        wout_c = np.ascontiguousarray(
            w_out[g * G * DH:(g + 1) * G * DH, :]).astype(ml_dtypes.bfloat16)
        in_maps.append({
            "xT": np.ascontiguousarray(x[b].T).astype(ml_dtypes.bfloat16),
            "wqkv": wqkv_c.astype(ml_dtypes.bfloat16),
            "wout": wout_c,
            "cos2": cos2.astype(ml_dtypes.bfloat16),
            "sin2": sin2.astype(ml_dtypes.bfloat16),
            "p2t": p2t.astype(ml_dtypes.bfloat16),
        })

    nc = _build()
    res = run_bass_kernel_spmd(nc, in_maps, core_ids=list(range(N_CORES)),
                               trace=_trace)
    out = np.empty((B, N, DIM), dtype=np.float32)
    for b in range(B):
        acc = res.results[G * b]["part"].astype(np.float32)
        for g in range(1, G):
            acc += res.results[G * b + g]["part"].astype(np.float32)
        out[b] = acc + b_out
    if _trace:
        kernel.last_results = res
    return out


# revision 51
# speedup vs baseline: 1.0027x; 1.0027x over previous
"""Trainium2 Bass kernel for nn_Attention_35021163332119.

Full multi-head attention: qkv = x @ w_qkv; RoPE(q, k); softmax(q k^T / sqrt(dh)) v;
out = heads @ w_out + b_out.  B=2, N=2048, DIM=1024, H=16, DH=64.

Sharding: 8 cores = (batch b in {0,1}) x (head-group g in {0..3} of 4 heads).
Each core computes its 4 heads end-to-end plus the partial output projection
for its head-group's rows of w_out; the host sums the 4 partials per batch
and adds b_out.

The schedule is built around the Activation engine: softmax exp is
Act-only (1 elem/cycle/partition at 1.2GHz -> ~129us for the 16.8M
scores/core) and nothing else can run it, so the kernel starts the exp
stream as early as possible and hides ALL other PE work inside the
Act-paced attention window (~262us baseline -> ~228us):
- Inputs DMA in consumption order (xT/w k-tiles first, split halves);
  pair-0 QKV runs k-OUTER so matmuls start when the first xT k-tile
  lands; RoPE is applied per 512-column chunk behind each pass.
- attention(0,0) begins once pair-0 RoPE and v0-7 are done (~45us, the
  phase-1 critical path); v8-15, ALL pair-1 QKV (as k-quarter jobs with
  an SBUF staging add) + its RoPE, and the output projection run as
  "filler" jobs woven one-per-j-iteration into the PE slack of later
  blocks.  Fillers never defer work a block reads (k columns are needed
  by every block's j-loop; proj(iq) lags >= 2 blocks past normalize).
- Scalar engine: exp only during the window (phase-1 v copies ride on it
  while idle); all other PSUM->SBUF copies on DVE.
- Scores matmul pairs carry tile_position (0,0)/(64,0) — the K=64 pair
  co-issues into disjoint PE row groups when PSUM frees line up.
- q/k tiles are bf16 (halves SBUF, 64-row weight loads); the partial
  output is written bf16 (halves the output DMA).  rel err ~1.1e-2 vs
  the 2e-2 gate (bf16 q/k contributes ~5e-3 over the f32r variant).
- PSUM: 3x[128,1024] score/filler bufs + 2x[65,512] PV accumulators = 8
  banks.  Fewer score bufs puts semaphore latency on the exp critical
  path (+40us); separate filler banks starve the score pipeline.
"""

import numpy as np

B, N, DIM, H, DH = 2, 2048, 1024, 16, 64
ROPE_BASE = 10000.0
SCALE = DH ** -0.5
N_CORES = 8
G = 4                 # heads per core
KT = DIM // 128       # contraction tiles
NT = N // 128         # sequence tiles

_cache = {}


def _rope_tables():
    inv_freq = (1.0 / (ROPE_BASE ** (np.arange(0, DH, 2, dtype=np.float32) / DH)))
    t = np.arange(N, dtype=np.float32)
    freqs = t[:, None] * inv_freq[None, :]          # [N, DH/2]
    freqs = np.repeat(freqs, 2, axis=-1)            # [N, DH] interleaved
    cosT = np.cos(freqs).T.astype(np.float32)       # [DH, N]
    sinT = np.sin(freqs).T.astype(np.float32)
    cos2 = np.concatenate([cosT, cosT], axis=0)     # [128, N] two heads stacked
    sin2 = np.concatenate([sinT, sinT], axis=0)
    return np.ascontiguousarray(cos2), np.ascontiguousarray(sin2)


def _p2t():
    # rot = P2 @ qT with P2 = blockdiag(P, P), P[2t, 2t+1] = -1, P[2t+1, 2t] = 1
    # matmul computes lhsT.T @ rhs, so pass P2.T
    p = np.zeros((DH, DH), dtype=np.float32)
    for t in range(DH // 2):
        p[2 * t, 2 * t + 1] = -1.0
        p[2 * t + 1, 2 * t] = 1.0
    p2 = np.zeros((128, 128), dtype=np.float32)
    p2[:DH, :DH] = p
    p2[DH:, DH:] = p
    return np.ascontiguousarray(p2.T)


def _build():
    if "nc" in _cache:
        return _cache["nc"]

    import concourse.mybir as mybir
    import concourse.tile as tile
    from concourse import bacc

    F32 = mybir.dt.float32
    BF16 = mybir.dt.bfloat16
    EXP = mybir.ActivationFunctionType.Exp

    nc = bacc.Bacc("TRN2", target_bir_lowering=False, debug=False)
    xT_d = nc.dram_tensor("xT", [DIM, N], BF16, kind="ExternalInput")
    w_d = nc.dram_tensor("wqkv", [DIM, 768], BF16, kind="ExternalInput")
    wout_d = nc.dram_tensor("wout", [G * DH, DIM], BF16, kind="ExternalInput")
    cos_d = nc.dram_tensor("cos2", [128, N], BF16, kind="ExternalInput")
    sin_d = nc.dram_tensor("sin2", [128, N], BF16, kind="ExternalInput")
    p2t_d = nc.dram_tensor("p2t", [128, 128], BF16, kind="ExternalInput")
    part_d = nc.dram_tensor("part", [N, DIM], BF16, kind="ExternalOutput")

    with tile.TileContext(nc) as tc:
        with tc.tile_pool(name="persist", bufs=1) as persist, \
             tc.tile_pool(name="att", bufs=5) as att, \
             tc.tile_pool(name="norm_w", bufs=2) as norm_w, \
             tc.tile_pool(name="outp", bufs=3) as outp, \
             tc.tile_pool(name="xph", bufs=1) as xph, \
             tc.tile_pool(name="rope_w", bufs=2) as rope_w, \
             tc.tile_pool(name="ps", bufs=3, space="PSUM") as ps, \
             tc.tile_pool(name="pso", bufs=2, space="PSUM") as pso:

            # ---- persistent tiles ----
            qk_sb = [persist.tile([128, N], BF16, tag=f"qk{m}", name=f"qk{m}")
                     for m in range(4)]          # q01T, q23T, k01T, k23T
            v_aug = persist.tile([128, NT, G, DH + 1], BF16, tag="vaug")
            wout_sb = [persist.tile([128, DIM], BF16, tag=f"wo{kk}", name=f"wo{kk}")
                       for kk in range(2)]
            outT = [persist.tile([128, N], BF16, tag=f"outT{p}", name=f"outT{p}")
                    for p in range(2)]

            # ---- phase-1 tiles ----
            xT = [xph.tile([128, N], BF16, tag=f"xT{k}", name=f"xT{k}")
                  for k in range(KT)]
            wqkv = [xph.tile([128, 768], BF16, tag=f"wq{k}", name=f"wq{k}")
                    for k in range(KT)]
            cos2 = xph.tile([128, N], BF16, tag="cos2")
            sin2 = xph.tile([128, N], BF16, tag="sin2")
            p2t = xph.tile([128, 128], BF16, tag="p2t")
            ones_col = xph.tile([128, NT, G, 1], F32, tag="ones")

            # ---- input DMA in consumption order ----
            xT_r = xT_d.ap().rearrange("(t p) n -> t p n", p=128)
            w_r = w_d.ap().rearrange("(t p) m -> t p m", p=128)
            for k in range(KT):
                nc.sync.dma_start(out=xT[k][:, 0:1024], in_=xT_r[k][:, 0:1024])
                nc.sync.dma_start(out=wqkv[k], in_=w_r[k])
            nc.sync.dma_start(out=cos2[:, 0:1024], in_=cos_d.ap()[:, 0:1024])
            nc.sync.dma_start(out=sin2[:, 0:1024], in_=sin_d.ap()[:, 0:1024])
            nc.sync.dma_start(out=p2t, in_=p2t_d.ap())
            for k in range(KT):
                nc.sync.dma_start(out=xT[k][:, 1024:2048],
                                  in_=xT_r[k][:, 1024:2048])
            nc.sync.dma_start(out=cos2[:, 1024:2048],
                              in_=cos_d.ap()[:, 1024:2048])
            nc.sync.dma_start(out=sin2[:, 1024:2048],
                              in_=sin_d.ap()[:, 1024:2048])
            for kk in range(2):
                nc.sync.dma_start(
                    out=wout_sb[kk],
                    in_=wout_d.ap().rearrange("(t p) m -> t p m", p=128)[kk])
            nc.vector.memset(ones_col, 1.0)

            def qk_pass0(cpass, ms=(0, 2)):
                """k-outer accumulation of pair-0 q/k tiles for one 1024-wide
                column chunk; consumes each xT k-tile as its DMA lands."""
                csl = slice(cpass * 1024, (cpass + 1) * 1024)
                psq = [ps.tile([128, 1024], F32, tag="s", name=f"qk_ps{m2}")
                       for m2 in range(len(ms))]
                for k in range(KT):
                    for m2, m in enumerate(ms):
                        for half in range(2):
                            hsl = slice(half * 512, (half + 1) * 512)
                            xsl = slice(cpass * 1024 + half * 512,
                                        cpass * 1024 + (half + 1) * 512)
                            nc.tensor.matmul(
                                psq[m2][:, hsl],
                                wqkv[k][:, m * 128:(m + 1) * 128],
                                xT[k][:, xsl],
                                start=(k == 0), stop=(k == KT - 1))
                for m2, m in enumerate(ms):
                    nc.vector.tensor_copy(qk_sb[m][:, csl], psq[m2])

            def rope_chunk(m, c):
                """RoPE on one 512-col chunk of qk_sb[m]: rotate-half via the
                PE permutation matmul, then combine with cos/sin on DVE."""
                csl = slice(c * 512, (c + 1) * 512)
                rp = ps.tile([128, 1024], F32, tag="s", name="rot")
                rps = rp[:, 0:512]
                nc.tensor.matmul(rps, p2t, qk_sb[m][:, csl],
                                 start=True, stop=True)
                tmp = rope_w.tile([128, 512], BF16, tag="rtmp")
                nc.vector.tensor_mul(tmp, rps, sin2[:, csl])
                nc.vector.tensor_mul(qk_sb[m][:, csl], qk_sb[m][:, csl],
                                     cos2[:, csl])
                nc.vector.tensor_add(qk_sb[m][:, csl], qk_sb[m][:, csl], tmp)

            def v_group(tn, cp_act=False):
                mm_ps = ps.tile([128, 1024], F32, tag="s", name="mm_v")
                for k in range(KT):
                    nc.tensor.matmul(
                        mm_ps[:, 0:G * DH],
                        xT[k][:, tn * 128:(tn + 1) * 128],
                        wqkv[k][:, 512:768],
                        start=(k == 0), stop=(k == KT - 1))
                src = mm_ps[:, 0:G * DH].rearrange("p (h d) -> p h d", h=G)
                if cp_act:
                    nc.scalar.copy(v_aug[:, tn, :, 0:DH], src)
                else:
                    nc.vector.tensor_copy(v_aug[:, tn, :, 0:DH], src)
                nc.vector.tensor_copy(v_aug[:, tn, :, DH:DH + 1],
                                      ones_col[:, tn])

            # pair-1 QKV as filler jobs: k-QUARTER per (m2, 512-col chunk) so
            # each job holds a PSUM buf only ~1us; halves are combined via an
            # SBUF staging tile and a DVE add.
            p1_stage = {}

            def pass1_q(m2, chunk, q):
                m = 1 if m2 == 0 else 3
                csl = slice(chunk * 512, (chunk + 1) * 512)
                tt = ps.tile([128, 1024], F32, tag="s", name="p1")
                t = tt[:, 0:512]
                for k in range(q * 4, q * 4 + 4):
                    nc.tensor.matmul(
                        t,
                        wqkv[k][:, m * 128:(m + 1) * 128], xT[k][:, csl],
                        start=(k == q * 4), stop=(k == q * 4 + 3))
                if q == 0:
                    st = rope_w.tile([128, 512], F32, tag="p1st")
                    p1_stage[(m2, chunk)] = st
                    nc.vector.tensor_copy(st, t)
                else:
                    st = p1_stage.pop((m2, chunk))
                    nc.vector.tensor_add(qk_sb[m][:, csl], st, t)

            def normalize(p, iq, o_ps):
                """PV accumulators -> normalized bf16 rows of outT.  The two
                head chains are interleaved stage-by-stage so their DMA and
                gpsimd latencies overlap instead of serializing."""
                isl = slice(iq * 512, iq * 512 + 512)
                o_sb = []
                recips = []
                bcs = []
                for hh in range(2):
                    t = norm_w.tile([DH + 1, 512], F32, tag=f"osb{hh}",
                                    name=f"osb{hh}")
                    nc.vector.tensor_copy(t, o_ps[hh])
                    o_sb.append(t)
                for hh in range(2):
                    r = norm_w.tile([1, 512], F32, tag=f"r0{hh}",
                                    name=f"r0{hh}")
                    nc.sync.dma_start(out=r, in_=o_sb[hh][DH:DH + 1, :])
                    recips.append(r)
                for hh in range(2):
                    nc.vector.reciprocal_approx_fast(recips[hh], recips[hh])
                for hh in range(2):
                    bc = norm_w.tile([DH, 512], F32, tag=f"bc{hh}",
                                     name=f"bc{hh}")
                    nc.gpsimd.partition_broadcast(bc, recips[hh])
                    bcs.append(bc)
                nc.vector.tensor_mul(outT[p][0:DH, isl],
                                     o_sb[0][0:DH, :], bcs[0])
                tmpb = norm_w.tile([DH, 512], BF16, tag="tmpb")
                nc.vector.tensor_mul(tmpb, o_sb[1][0:DH, :], bcs[1])
                nc.sync.dma_start(out=outT[p][DH:2 * DH, isl], in_=tmpb)

            def emit_pv(p, o_ps, jj, exps):
                for hh in range(2):
                    for half in range(2):
                        j = 2 * jj + half
                        nc.tensor.matmul(
                            o_ps[hh],
                            v_aug[:, j, 2 * p + hh, :],
                            exps[hh][:, half * 512:(half + 1) * 512],
                            start=(j == 0), stop=(j == NT - 1))

            def run_attention(blocks):
                """All attention blocks as ONE continuous scores -> exp -> PV
                pipeline.  PV lags exp by one iteration ACROSS block
                boundaries, so each next block's first scores sit between a
                block's last exp and its last PV in PE program order — the
                exp stream never waits out a block tail.  Filler jobs pop
                into PE slack at the given j-slots."""
                pend = None      # (p, iq, o_ps, jj, exps)
                for p, iq, fillers, slots in blocks:
                    o_ps = [pso.tile([DH + 1, 512], F32, tag="o",
                                     name=f"o{hh}") for hh in range(2)]
                    qT = qk_sb[p]
                    kTt = qk_sb[2 + p]
                    isl = slice(iq * 512, iq * 512 + 512)
                    fillers = list(fillers)
                    if slots is None:
                        slots = list(range(len(fillers)))
                    fi = 0
                    for jj in range(NT // 2):
                        s_ps = [ps.tile([128, 1024], F32, tag="s",
                                        name=f"s{hh}") for hh in range(2)]
                        for half in range(2):
                            j = 2 * jj + half
                            jsl = slice(j * 128, (j + 1) * 128)
                            for hh in range(2):
                                hsl = slice(hh * DH, (hh + 1) * DH)
                                # K=64 pair in disjoint PE row groups can
                                # co-issue (explicit tile_position)
                                nc.tensor.matmul(
                                    s_ps[hh][:, half * 512:(half + 1) * 512],
                                    kTt[hsl, jsl], qT[hsl, isl],
                                    start=True, stop=True,
                                    tile_position=(hh * DH, 0))
                        exps = []
                        for hh in range(2):
                            expT = att.tile([128, 1024], BF16, tag="exp")
                            nc.scalar.activation(expT, s_ps[hh], EXP,
                                                 scale=SCALE)
                            exps.append(expT)
                        if pend is not None:
                            pp, piq, po, pjj, pexps = pend
                            emit_pv(pp, po, pjj, pexps)
                            if pjj == NT // 2 - 1:
                                normalize(pp, piq, po)
                        pend = (p, iq, o_ps, jj, exps)
                        while fi < len(fillers) and fi < len(slots) \
                                and slots[fi] <= jj:
                            fillers[fi]()
                            fi += 1
                    while fi < len(fillers):
                        fillers[fi]()
                        fi += 1
                pp, piq, po, pjj, pexps = pend
                emit_pv(pp, po, pjj, pexps)
                normalize(pp, piq, po)

            def proj_tile(tn, cp_act=False):
                nsl = slice(tn * 128, (tn + 1) * 128)
                out_sb = outp.tile([128, DIM], BF16, tag="osb")
                f_ps = ps.tile([128, 1024], F32, tag="s", name="f_ps")
                for c2 in range(2):
                    c2sl = slice(c2 * 512, (c2 + 1) * 512)
                    for kk in range(2):
                        nc.tensor.matmul(
                            f_ps[:, c2sl],
                            outT[kk][:, nsl], wout_sb[kk][:, c2sl],
                            start=(kk == 0), stop=(kk == 1))
                if cp_act:
                    nc.scalar.copy(out_sb, f_ps)
                else:
                    nc.vector.tensor_copy(out_sb, f_ps)
                nc.sync.dma_start(
                    out=part_d.ap().rearrange("(t p) m -> t p m", p=128)[tn],
                    in_=out_sb)

            # ---- emission order ----
            # Inline phase 1 is the minimum needed by attention(0,0):
            # pair-0 qk for columns 0:1024 (k-outer, consumes xT as the DMA
            # lands), its rope, and v tiles 0-7 (copies on the idle scalar
            # engine).  Everything else — pair-0 columns 1024:2048, ALL of
            # pair-1 qkv+rope, v8-15, and the projection — is filler jobs
            # inside the Act-paced attention window.
            qk_pass0(0)
            for m in (0, 2):
                for c in (0, 1):
                    rope_chunk(m, c)
            for tn in range(0, 8):
                v_group(tn, cp_act=True)
            qk_pass0(1)
            for m in (0, 2):
                for c in (2, 3):
                    rope_chunk(m, c)

            def F(fn, *a):
                return lambda: fn(*a)

            # Deadlines: attention(p, iq) scores at jj=4/6 read k-chunks
            # 2/3 (filler+rope one block earlier or early-slot same block);
            # attention(1,*) reads roped qk_sb[1]/[3] produced in the
            # (0,2)/(0,3) blocks.  proj(iq) lags >= 2 blocks.
            f_v = [F(v_group, tn) for tn in range(8, 16)]
            f_p1 = [F(pass1_q, m2, c, q)
                    for m2 in range(2) for c in range(4) for q in range(2)]
            f_r1 = [F(rope_chunk, m, c) for m in (1, 3) for c in range(4)]

            run_attention([
                # v fillers start at slot 1: a slot-0 filler lands exactly in
                # the score-pipeline fill of the first block (v15 runs as the
                # post-loop leftover, still ahead of the cross-block PV(7))
                (0, 0, f_v, [1, 2, 3, 4, 5, 6, 7]),
                (0, 1, f_p1[0:8], None),
                (0, 2, f_p1[8:16], None),
                (0, 3, f_r1, None),
                (1, 0, [], None),
                (1, 1, [F(proj_tile, tn) for tn in range(0, 4)],
                 [4, 5, 6, 7]),
                (1, 2, [F(proj_tile, tn) for tn in range(4, 8)],
                 [4, 5, 6, 7]),
                (1, 3, [F(proj_tile, tn) for tn in range(8, 10)], [5, 7]),
            ])
            # proj 10,11 depend only on normalize(1,2): they keep the PE warm
            # while normalize(1,3) drains; 12-15 follow it, staged into one
            # SBUF tile and written with a single DMA (one issue+sem instead
            # of four on the critical tail).
            for tn in range(10, 12):
                proj_tile(tn)
            tail_sb = persist.tile([128, 4, DIM], BF16, tag="tail")
            for ti, tn in enumerate(range(12, 16)):
                nsl = slice(tn * 128, (tn + 1) * 128)
                f_ps = ps.tile([128, 1024], F32, tag="s", name="f_ps")
                for c2 in range(2):
                    c2sl = slice(c2 * 512, (c2 + 1) * 512)
                    for kk in range(2):
                        nc.tensor.matmul(
                            f_ps[:, c2sl],
                            outT[kk][:, nsl], wout_sb[kk][:, c2sl],
                            start=(kk == 0), stop=(kk == 1))
                if tn % 2 == 0:
                    nc.scalar.copy(tail_sb[:, ti], f_ps)
                else:
                    nc.vector.tensor_copy(tail_sb[:, ti], f_ps)
            nc.sync.dma_start(
                out=part_d.ap().rearrange("(t p) m -> p t m", p=128)[:, 12:16],
                in_=tail_sb)
    nc.compile()
    _cache["nc"] = nc
    return nc


def kernel(x, w_qkv, w_out, b_out, _trace=False):
    import ml_dtypes
    from concourse.bass_utils import run_bass_kernel_spmd

    x = np.asarray(x, dtype=np.float32)
    w_qkv = np.asarray(w_qkv, dtype=np.float32)
    w_out = np.asarray(w_out, dtype=np.float32)
    b_out = np.asarray(b_out, dtype=np.float32)

    cos2, sin2 = _rope_tables()
    p2t = _p2t()

    in_maps = []
    for c in range(N_CORES):
        b, g = divmod(c, G)
        cols = []
        for blk in range(2):                      # q block, k block
            base = blk * H * DH + g * G * DH
            cols.append(w_qkv[:, base:base + G * DH])
        cols.append(w_qkv[:, 2 * H * DH + g * G * DH:
                          2 * H * DH + (g + 1) * G * DH])   # v block
        wqkv_c = np.ascontiguousarray(np.concatenate(cols, axis=1))  # [DIM,768]
        wout_c = np.ascontiguousarray(
            w_out[g * G * DH:(g + 1) * G * DH, :]).astype(ml_dtypes.bfloat16)
        in_maps.append({
            "xT": np.ascontiguousarray(x[b].T).astype(ml_dtypes.bfloat16),
            "wqkv": wqkv_c.astype(ml_dtypes.bfloat16),
            "wout": wout_c,
            "cos2": cos2.astype(ml_dtypes.bfloat16),
            "sin2": sin2.astype(ml_dtypes.bfloat16),
            "p2t": p2t.astype(ml_dtypes.bfloat16),
        })

    nc = _build()
    res = run_bass_kernel_spmd(nc, in_maps, core_ids=list(range(N_CORES)),
                               trace=_trace)
    out = np.empty((B, N, DIM), dtype=np.float32)
    for b in range(B):
        acc = res.results[G * b]["part"].astype(np.float32)
        for g in range(1, G):
            acc += res.results[G * b + g]["part"].astype(np.float32)
        out[b] = acc + b_out
    if _trace:
        kernel.last_results = res
    return out


# revision 53
# speedup vs baseline: 1.1494x; 1.1463x over previous
"""Trainium2 Bass kernel for nn_Attention_35021163332119.

Full multi-head attention: qkv = x @ w_qkv; RoPE(q, k); softmax(q k^T / sqrt(dh)) v;
out = heads @ w_out + b_out.  B=2, N=2048, DIM=1024, H=16, DH=64.

Sharding: 8 cores = (batch b in {0,1}) x (head-group g in {0..3} of 4 heads).
Each core computes its 4 heads end-to-end plus the partial output projection
for its head-group's rows of w_out; the host sums the 4 partials per batch
and adds b_out.

The schedule is built around the Activation engine: softmax exp is
Act-only (1 elem/cycle/partition at 1.2GHz -> ~129us for the 16.8M
scores/core) and nothing else can run it, so the kernel starts the exp
stream as early as possible and hides ALL other PE work inside the
Act-paced attention window (~262us baseline -> ~228us):
- Inputs DMA in consumption order (xT/w k-tiles first, split halves);
  pair-0 QKV runs k-OUTER so matmuls start when the first xT k-tile
  lands; RoPE is applied per 512-column chunk behind each pass.
- attention(0,0) begins once pair-0 RoPE and v0-7 are done (~45us, the
  phase-1 critical path); v8-15, ALL pair-1 QKV (as k-quarter jobs with
  an SBUF staging add) + its RoPE, and the output projection run as
  "filler" jobs woven one-per-j-iteration into the PE slack of later
  blocks.  Fillers never defer work a block reads (k columns are needed
  by every block's j-loop; proj(iq) lags >= 2 blocks past normalize).
- Scalar engine: exp only during the window (phase-1 v copies ride on it
  while idle); all other PSUM->SBUF copies on DVE.
- Scores matmul pairs carry tile_position (0,0)/(64,0) — the K=64 pair
  co-issues into disjoint PE row groups when PSUM frees line up.
- q/k tiles are bf16 (halves SBUF, 64-row weight loads); the partial
  output is written bf16 (halves the output DMA).  rel err ~1.1e-2 vs
  the 2e-2 gate (bf16 q/k contributes ~5e-3 over the f32r variant).
- PSUM: 3x[128,1024] score/filler bufs + 2x[65,512] PV accumulators = 8
  banks.  Fewer score bufs puts semaphore latency on the exp critical
  path (+40us); separate filler banks starve the score pipeline.
"""

import numpy as np

B, N, DIM, H, DH = 2, 2048, 1024, 16, 64
ROPE_BASE = 10000.0
SCALE = DH ** -0.5
N_CORES = 8
G = 4                 # heads per core
KT = DIM // 128       # contraction tiles
NT = N // 128         # sequence tiles

_cache = {}


def _rope_tables():
    inv_freq = (1.0 / (ROPE_BASE ** (np.arange(0, DH, 2, dtype=np.float32) / DH)))
    t = np.arange(N, dtype=np.float32)
    freqs = t[:, None] * inv_freq[None, :]          # [N, DH/2]
    freqs = np.repeat(freqs, 2, axis=-1)            # [N, DH] interleaved
    cosT = np.cos(freqs).T.astype(np.float32)       # [DH, N]
    sinT = np.sin(freqs).T.astype(np.float32)
    cos2 = np.concatenate([cosT, cosT], axis=0)     # [128, N] two heads stacked
    sin2 = np.concatenate([sinT, sinT], axis=0)
    return np.ascontiguousarray(cos2), np.ascontiguousarray(sin2)


def _p2t():
    # rot = P2 @ qT with P2 = blockdiag(P, P), P[2t, 2t+1] = -1, P[2t+1, 2t] = 1
    # matmul computes lhsT.T @ rhs, so pass P2.T
    p = np.zeros((DH, DH), dtype=np.float32)
    for t in range(DH // 2):
        p[2 * t, 2 * t + 1] = -1.0
        p[2 * t + 1, 2 * t] = 1.0
    p2 = np.zeros((128, 128), dtype=np.float32)
    p2[:DH, :DH] = p
    p2[DH:, DH:] = p
    return np.ascontiguousarray(p2.T)


def _build():
    if "nc" in _cache:
        return _cache["nc"]

    import concourse.mybir as mybir
    import concourse.tile as tile
    from concourse import bacc

    F32 = mybir.dt.float32
    BF16 = mybir.dt.bfloat16
    EXP = mybir.ActivationFunctionType.Exp

    nc = bacc.Bacc("TRN2", target_bir_lowering=False, debug=False)
    xT_d = nc.dram_tensor("xT", [DIM, N], BF16, kind="ExternalInput")
    w_d = nc.dram_tensor("wqkv", [DIM, 768], BF16, kind="ExternalInput")
    wout_d = nc.dram_tensor("wout", [G * DH, DIM], BF16, kind="ExternalInput")
    cos_d = nc.dram_tensor("cos2", [128, N], BF16, kind="ExternalInput")
    sin_d = nc.dram_tensor("sin2", [128, N], BF16, kind="ExternalInput")
    p2t_d = nc.dram_tensor("p2t", [128, 128], BF16, kind="ExternalInput")
    part_d = nc.dram_tensor("part", [N, DIM], BF16, kind="ExternalOutput")

    with tile.TileContext(nc) as tc:
        with tc.tile_pool(name="persist", bufs=1) as persist, \
             tc.tile_pool(name="att", bufs=5) as att, \
             tc.tile_pool(name="norm_w", bufs=2) as norm_w, \
             tc.tile_pool(name="outp", bufs=3) as outp, \
             tc.tile_pool(name="xph", bufs=1) as xph, \
             tc.tile_pool(name="rope_w", bufs=2) as rope_w, \
             tc.tile_pool(name="ps", bufs=3, space="PSUM") as ps, \
             tc.tile_pool(name="pso", bufs=2, space="PSUM") as pso:

            # ---- persistent tiles ----
            qk_sb = [persist.tile([128, N], BF16, tag=f"qk{m}", name=f"qk{m}")
                     for m in range(4)]          # q01T, q23T, k01T, k23T
            v_aug = persist.tile([128, NT, G, DH + 1], BF16, tag="vaug")
            wout_sb = [persist.tile([128, DIM], BF16, tag=f"wo{kk}", name=f"wo{kk}")
                       for kk in range(2)]
            outT = [persist.tile([128, N], BF16, tag=f"outT{p}", name=f"outT{p}")
                    for p in range(2)]

            # ---- phase-1 tiles ----
            xT = [xph.tile([128, N], BF16, tag=f"xT{k}", name=f"xT{k}")
                  for k in range(KT)]
            wqkv = [xph.tile([128, 768], BF16, tag=f"wq{k}", name=f"wq{k}")
                    for k in range(KT)]
            cos2 = xph.tile([128, N], BF16, tag="cos2")
            sin2 = xph.tile([128, N], BF16, tag="sin2")
            p2t = xph.tile([128, 128], BF16, tag="p2t")
            ones_col = xph.tile([128, NT, G, 1], F32, tag="ones")

            # ---- input DMA in consumption order ----
            xT_r = xT_d.ap().rearrange("(t p) n -> t p n", p=128)
            w_r = w_d.ap().rearrange("(t p) m -> t p m", p=128)
            for k in range(KT):
                nc.sync.dma_start(out=xT[k][:, 0:1024], in_=xT_r[k][:, 0:1024])
                nc.sync.dma_start(out=wqkv[k], in_=w_r[k])
            nc.sync.dma_start(out=cos2[:, 0:1024], in_=cos_d.ap()[:, 0:1024])
            nc.sync.dma_start(out=sin2[:, 0:1024], in_=sin_d.ap()[:, 0:1024])
            nc.sync.dma_start(out=p2t, in_=p2t_d.ap())
            for k in range(KT):
                nc.sync.dma_start(out=xT[k][:, 1024:2048],
                                  in_=xT_r[k][:, 1024:2048])
            nc.sync.dma_start(out=cos2[:, 1024:2048],
                              in_=cos_d.ap()[:, 1024:2048])
            nc.sync.dma_start(out=sin2[:, 1024:2048],
                              in_=sin_d.ap()[:, 1024:2048])
            for kk in range(2):
                nc.sync.dma_start(
                    out=wout_sb[kk],
                    in_=wout_d.ap().rearrange("(t p) m -> t p m", p=128)[kk])
            nc.vector.memset(ones_col, 1.0)

            def qk_pass0(cpass, ms=(0, 2)):
                """k-outer accumulation of pair-0 q/k tiles for one 1024-wide
                column chunk; consumes each xT k-tile as its DMA lands."""
                csl = slice(cpass * 1024, (cpass + 1) * 1024)
                psq = [ps.tile([128, 1024], F32, tag="s", name=f"qk_ps{m2}")
                       for m2 in range(len(ms))]
                for k in range(KT):
                    for m2, m in enumerate(ms):
                        for half in range(2):
                            hsl = slice(half * 512, (half + 1) * 512)
                            xsl = slice(cpass * 1024 + half * 512,
                                        cpass * 1024 + (half + 1) * 512)
                            nc.tensor.matmul(
                                psq[m2][:, hsl],
                                wqkv[k][:, m * 128:(m + 1) * 128],
                                xT[k][:, xsl],
                                start=(k == 0), stop=(k == KT - 1))
                for m2, m in enumerate(ms):
                    nc.vector.tensor_copy(qk_sb[m][:, csl], psq[m2])

            def rope_chunk(m, c):
                """RoPE on one 512-col chunk of qk_sb[m]: rotate-half via the
                PE permutation matmul, then combine with cos/sin on DVE."""
                csl = slice(c * 512, (c + 1) * 512)
                rp = ps.tile([128, 1024], F32, tag="s", name="rot")
                rps = rp[:, 0:512]
                nc.tensor.matmul(rps, p2t, qk_sb[m][:, csl],
                                 start=True, stop=True)
                tmp = rope_w.tile([128, 512], BF16, tag="rtmp")
                nc.vector.tensor_mul(tmp, rps, sin2[:, csl])
                nc.vector.tensor_mul(qk_sb[m][:, csl], qk_sb[m][:, csl],
                                     cos2[:, csl])
                nc.vector.tensor_add(qk_sb[m][:, csl], qk_sb[m][:, csl], tmp)

            def v_group(tn, cp_act=False):
                mm_ps = ps.tile([128, 1024], F32, tag="s", name="mm_v")
                for k in range(KT):
                    nc.tensor.matmul(
                        mm_ps[:, 0:G * DH],
                        xT[k][:, tn * 128:(tn + 1) * 128],
                        wqkv[k][:, 512:768],
                        start=(k == 0), stop=(k == KT - 1))
                src = mm_ps[:, 0:G * DH].rearrange("p (h d) -> p h d", h=G)
                if cp_act:
                    nc.scalar.copy(v_aug[:, tn, :, 0:DH], src)
                else:
                    nc.vector.tensor_copy(v_aug[:, tn, :, 0:DH], src)
                nc.vector.tensor_copy(v_aug[:, tn, :, DH:DH + 1],
                                      ones_col[:, tn])

            # pair-1 QKV as filler jobs: k-QUARTER per (m2, 512-col chunk) so
            # each job holds a PSUM buf only ~1us; halves are combined via an
            # SBUF staging tile and a DVE add.
            p1_stage = {}

            def pass1_q(m2, chunk, q):
                m = 1 if m2 == 0 else 3
                csl = slice(chunk * 512, (chunk + 1) * 512)
                tt = ps.tile([128, 1024], F32, tag="s", name="p1")
                t = tt[:, 0:512]
                for k in range(q * 4, q * 4 + 4):
                    nc.tensor.matmul(
                        t,
                        wqkv[k][:, m * 128:(m + 1) * 128], xT[k][:, csl],
                        start=(k == q * 4), stop=(k == q * 4 + 3))
                if q == 0:
                    st = rope_w.tile([128, 512], F32, tag="p1st")
                    p1_stage[(m2, chunk)] = st
                    nc.vector.tensor_copy(st, t)
                else:
                    st = p1_stage.pop((m2, chunk))
                    nc.vector.tensor_add(qk_sb[m][:, csl], st, t)

            def normalize(p, iq, o_ps):
                """PV accumulators -> normalized bf16 rows of outT.  The two
                head chains are interleaved stage-by-stage so their DMA and
                gpsimd latencies overlap instead of serializing."""
                isl = slice(iq * 512, iq * 512 + 512)
                o_sb = []
                recips = []
                bcs = []
                for hh in range(2):
                    t = norm_w.tile([DH + 1, 512], F32, tag=f"osb{hh}",
                                    name=f"osb{hh}")
                    nc.vector.tensor_copy(t, o_ps[hh])
                    o_sb.append(t)
                for hh in range(2):
                    r = norm_w.tile([1, 512], F32, tag=f"r0{hh}",
                                    name=f"r0{hh}")
                    nc.sync.dma_start(out=r, in_=o_sb[hh][DH:DH + 1, :])
                    recips.append(r)
                for hh in range(2):
                    nc.vector.reciprocal_approx_fast(recips[hh], recips[hh])
                for hh in range(2):
                    bc = norm_w.tile([DH, 512], F32, tag=f"bc{hh}",
                                     name=f"bc{hh}")
                    nc.gpsimd.partition_broadcast(bc, recips[hh])
                    bcs.append(bc)
                nc.vector.tensor_mul(outT[p][0:DH, isl],
                                     o_sb[0][0:DH, :], bcs[0])
                tmpb = norm_w.tile([DH, 512], BF16, tag="tmpb")
                nc.vector.tensor_mul(tmpb, o_sb[1][0:DH, :], bcs[1])
                nc.sync.dma_start(out=outT[p][DH:2 * DH, isl], in_=tmpb)

            def emit_pv(p, o_ps, jj, exps):
                for hh in range(2):
                    for half in range(2):
                        j = 2 * jj + half
                        nc.tensor.matmul(
                            o_ps[hh],
                            v_aug[:, j, 2 * p + hh, :],
                            exps[hh][:, half * 512:(half + 1) * 512],
                            start=(j == 0), stop=(j == NT - 1))

            def run_attention(blocks):
                """All attention blocks as ONE continuous scores -> exp -> PV
                pipeline.  PV lags exp by one iteration ACROSS block
                boundaries, so each next block's first scores sit between a
                block's last exp and its last PV in PE program order — the
                exp stream never waits out a block tail.  Filler jobs pop
                into PE slack at the given j-slots."""
                pend = None      # (p, iq, o_ps, jj, exps)
                for p, iq, fillers, slots in blocks:
                    o_ps = [pso.tile([DH + 1, 512], F32, tag="o",
                                     name=f"o{hh}") for hh in range(2)]
                    qT = qk_sb[p]
                    kTt = qk_sb[2 + p]
                    isl = slice(iq * 512, iq * 512 + 512)
                    fillers = list(fillers)
                    if slots is None:
                        slots = list(range(len(fillers)))
                    fi = 0
                    for jj in range(NT // 2):
                        s_ps = [ps.tile([128, 1024], F32, tag="s",
                                        name=f"s{hh}") for hh in range(2)]
                        for half in range(2):
                            j = 2 * jj + half
                            jsl = slice(j * 128, (j + 1) * 128)
                            for hh in range(2):
                                hsl = slice(hh * DH, (hh + 1) * DH)
                                # K=64 pair in disjoint PE row groups can
                                # co-issue (explicit tile_position)
                                nc.tensor.matmul(
                                    s_ps[hh][:, half * 512:(half + 1) * 512],
                                    kTt[hsl, jsl], qT[hsl, isl],
                                    start=True, stop=True,
                                    tile_position=(hh * DH, 0))
                        exps = []
                        for hh in range(2):
                            expT = att.tile([128, 1024], BF16, tag="exp")
                            nc.scalar.activation(expT, s_ps[hh], EXP,
                                                 scale=SCALE)
                            exps.append(expT)
                        if pend is not None:
                            pp, piq, po, pjj, pexps = pend
                            emit_pv(pp, po, pjj, pexps)
                            if pjj == NT // 2 - 1:
                                normalize(pp, piq, po)
                        pend = (p, iq, o_ps, jj, exps)
                        while fi < len(fillers) and fi < len(slots) \
                                and slots[fi] <= jj:
                            fillers[fi]()
                            fi += 1
                    while fi < len(fillers):
                        fillers[fi]()
                        fi += 1
                pp, piq, po, pjj, pexps = pend
                emit_pv(pp, po, pjj, pexps)
                normalize(pp, piq, po)

            def proj_tile(tn, cp_act=False):
                nsl = slice(tn * 128, (tn + 1) * 128)
                out_sb = outp.tile([128, DIM], BF16, tag="osb")
                f_ps = ps.tile([128, 1024], F32, tag="s", name="f_ps")
                for c2 in range(2):
                    c2sl = slice(c2 * 512, (c2 + 1) * 512)
                    for kk in range(2):
                        nc.tensor.matmul(
                            f_ps[:, c2sl],
                            outT[kk][:, nsl], wout_sb[kk][:, c2sl],
                            start=(kk == 0), stop=(kk == 1))
                if cp_act:
                    nc.scalar.copy(out_sb, f_ps)
                else:
                    nc.vector.tensor_copy(out_sb, f_ps)
                nc.sync.dma_start(
                    out=part_d.ap().rearrange("(t p) m -> t p m", p=128)[tn],
                    in_=out_sb)

            # ---- emission order ----
            # Inline phase 1 is the minimum needed by attention(0,0):
            # pair-0 qk for columns 0:1024 (k-outer, consumes xT as the DMA
            # lands), its rope, and v tiles 0-7 (copies on the idle scalar
            # engine).  Everything else — pair-0 columns 1024:2048, ALL of
            # pair-1 qkv+rope, v8-15, and the projection — is filler jobs
            # inside the Act-paced attention window.
            qk_pass0(0)
            for m in (0, 2):
                for c in (0, 1):
                    rope_chunk(m, c)
            for tn in range(0, 8):
                v_group(tn, cp_act=True)
            qk_pass0(1)
            for m in (0, 2):
                for c in (2, 3):
                    rope_chunk(m, c)

            def F(fn, *a):
                return lambda: fn(*a)

            # Deadlines: attention(p, iq) scores at jj=4/6 read k-chunks
            # 2/3 (filler+rope one block earlier or early-slot same block);
            # attention(1,*) reads roped qk_sb[1]/[3] produced in the
            # (0,2)/(0,3) blocks.  proj(iq) lags >= 2 blocks.
            # v8-15 copies ride the scalar engine: during att(0,0) the DVE is
            # busy with the pair-0 c2/c3 rope combines, so DVE copies would
            # hold the score PSUM bufs and starve the exp feed (~7.5us of
            # gaps); the ~0.4us Act copies cost less than the stalls.
            f_v = [F(v_group, tn, True) for tn in range(8, 16)]
            f_p1 = [F(pass1_q, m2, c, q)
                    for m2 in range(2) for c in range(4) for q in range(2)]
            f_r1 = [F(rope_chunk, m, c) for m in (1, 3) for c in range(4)]

            run_attention([
                # v fillers start at slot 1: a slot-0 filler lands exactly in
                # the score-pipeline fill of the first block (v15 runs as the
                # post-loop leftover, still ahead of the cross-block PV(7))
                (0, 0, f_v, [1, 2, 3, 4, 5, 6, 7]),
                (0, 1, f_p1[0:8], None),
                (0, 2, f_p1[8:16], None),
                (0, 3, f_r1, None),
                (1, 0, [], None),
                (1, 1, [F(proj_tile, tn) for tn in range(0, 4)],
                 [4, 5, 6, 7]),
                (1, 2, [F(proj_tile, tn) for tn in range(4, 8)],
                 [3, 4, 5, 6]),
                (1, 3, [F(proj_tile, tn) for tn in range(8, 10)], [3, 5]),
            ])
            # proj 10,11 depend only on normalize(1,2): they keep the PE warm
            # while normalize(1,3) drains; 12-15 follow it, staged into one
            # SBUF tile and written with a single DMA (one issue+sem instead
            # of four on the critical tail).
            for tn in range(10, 12):
                proj_tile(tn)
            tail_sb = persist.tile([128, 4, DIM], BF16, tag="tail")
            for ti, tn in enumerate(range(12, 16)):
                nsl = slice(tn * 128, (tn + 1) * 128)
                f_ps = ps.tile([128, 1024], F32, tag="s", name="f_ps")
                for c2 in range(2):
                    c2sl = slice(c2 * 512, (c2 + 1) * 512)
                    for kk in range(2):
                        nc.tensor.matmul(
                            f_ps[:, c2sl],
                            outT[kk][:, nsl], wout_sb[kk][:, c2sl],
                            start=(kk == 0), stop=(kk == 1))
                if tn % 2 == 0:
                    nc.scalar.copy(tail_sb[:, ti], f_ps)
                else:
                    nc.vector.tensor_copy(tail_sb[:, ti], f_ps)
            nc.sync.dma_start(
                out=part_d.ap().rearrange("(t p) m -> p t m", p=128)[:, 12:16],
                in_=tail_sb)
    nc.compile()
    _cache["nc"] = nc
    return nc


def kernel(x, w_qkv, w_out, b_out, _trace=False):
    import ml_dtypes
    from concourse.bass_utils import run_bass_kernel_spmd

    x = np.asarray(x, dtype=np.float32)
    w_qkv = np.asarray(w_qkv, dtype=np.float32)
    w_out = np.asarray(w_out, dtype=np.float32)
    b_out = np.asarray(b_out, dtype=np.float32)

    cos2, sin2 = _rope_tables()
    p2t = _p2t()

    in_maps = []
    for c in range(N_CORES):
        b, g = divmod(c, G)
        cols = []
        for blk in range(2):                      # q block, k block
            base = blk * H * DH + g * G * DH
            cols.append(w_qkv[:, base:base + G * DH])
        cols.append(w_qkv[:, 2 * H * DH + g * G * DH:
                          2 * H * DH + (g + 1) * G * DH])   # v block
        wqkv_c = np.ascontiguousarray(np.concatenate(cols, axis=1))  # [DIM,768]
        wout_c = np.ascontiguousarray(
            w_out[g * G * DH:(g + 1) * G * DH, :]).astype(ml_dtypes.bfloat16)
        in_maps.append({
            "xT": np.ascontiguousarray(x[b].T).astype(ml_dtypes.bfloat16),
            "wqkv": wqkv_c.astype(ml_dtypes.bfloat16),
            "wout": wout_c,
            "cos2": cos2.astype(ml_dtypes.bfloat16),
            "sin2": sin2.astype(ml_dtypes.bfloat16),
            "p2t": p2t.astype(ml_dtypes.bfloat16),
        })

    nc = _build()
    res = run_bass_kernel_spmd(nc, in_maps, core_ids=list(range(N_CORES)),
                               trace=_trace)
    out = np.empty((B, N, DIM), dtype=np.float32)
    for b in range(B):
        acc = res.results[G * b]["part"].astype(np.float32)
        for g in range(1, G):
            acc += res.results[G * b + g]["part"].astype(np.float32)
        out[b] = acc + b_out
    if _trace:
        kernel.last_results = res
    return out


# revision 56
# speedup vs baseline: 1.1774x; 1.0243x over previous
"""Trainium2 Bass kernel for nn_Attention_35021163332119.

Full multi-head attention: qkv = x @ w_qkv; RoPE(q, k); softmax(q k^T / sqrt(dh)) v;
out = heads @ w_out + b_out.  B=2, N=2048, DIM=1024, H=16, DH=64.

Sharding: 8 cores = (batch b in {0,1}) x (head-group g in {0..3} of 4 heads).
Each core computes its 4 heads end-to-end plus the partial output projection
for its head-group's rows of w_out; the host sums the 4 partials per batch
and adds b_out.

The schedule is built around the Activation engine: softmax exp is
Act-only (1 elem/cycle/partition at 1.2GHz -> ~129us for the 16.8M
scores/core) and nothing else can run it, so the kernel starts the exp
stream as early as possible and hides ALL other PE work inside the
Act-paced attention window (~262us baseline -> ~228us):
- Inputs DMA in consumption order (xT/w k-tiles first, split halves);
  pair-0 QKV runs k-OUTER so matmuls start when the first xT k-tile
  lands; RoPE is applied per 512-column chunk behind each pass.
- attention(0,0) begins once pair-0 RoPE and v0-7 are done (~45us, the
  phase-1 critical path); v8-15, ALL pair-1 QKV (as k-quarter jobs with
  an SBUF staging add) + its RoPE, and the output projection run as
  "filler" jobs woven one-per-j-iteration into the PE slack of later
  blocks.  Fillers never defer work a block reads (k columns are needed
  by every block's j-loop; proj(iq) lags >= 2 blocks past normalize).
- Scalar engine: exp only during the window (phase-1 v copies ride on it
  while idle); all other PSUM->SBUF copies on DVE.
- Scores matmul pairs carry tile_position (0,0)/(64,0) — the K=64 pair
  co-issues into disjoint PE row groups when PSUM frees line up.
- q/k tiles are bf16 (halves SBUF, 64-row weight loads); the partial
  output is written bf16 (halves the output DMA).  rel err ~1.1e-2 vs
  the 2e-2 gate (bf16 q/k contributes ~5e-3 over the f32r variant).
- PSUM: 3x[128,1024] score/filler bufs + 2x[65,512] PV accumulators = 8
  banks.  Fewer score bufs puts semaphore latency on the exp critical
  path (+40us); separate filler banks starve the score pipeline.
"""

import numpy as np

B, N, DIM, H, DH = 2, 2048, 1024, 16, 64
ROPE_BASE = 10000.0
SCALE = DH ** -0.5
N_CORES = 8
G = 4                 # heads per core
KT = DIM // 128       # contraction tiles
NT = N // 128         # sequence tiles

_cache = {}


def _rope_tables():
    inv_freq = (1.0 / (ROPE_BASE ** (np.arange(0, DH, 2, dtype=np.float32) / DH)))
    t = np.arange(N, dtype=np.float32)
    freqs = t[:, None] * inv_freq[None, :]          # [N, DH/2]
    freqs = np.repeat(freqs, 2, axis=-1)            # [N, DH] interleaved
    cosT = np.cos(freqs).T.astype(np.float32)       # [DH, N]
    sinT = np.sin(freqs).T.astype(np.float32)
    cos2 = np.concatenate([cosT, cosT], axis=0)     # [128, N] two heads stacked
    sin2 = np.concatenate([sinT, sinT], axis=0)
    return np.ascontiguousarray(cos2), np.ascontiguousarray(sin2)


def _p2t():
    # rot = P2 @ qT with P2 = blockdiag(P, P), P[2t, 2t+1] = -1, P[2t+1, 2t] = 1
    # matmul computes lhsT.T @ rhs, so pass P2.T
    p = np.zeros((DH, DH), dtype=np.float32)
    for t in range(DH // 2):
        p[2 * t, 2 * t + 1] = -1.0
        p[2 * t + 1, 2 * t] = 1.0
    p2 = np.zeros((128, 128), dtype=np.float32)
    p2[:DH, :DH] = p
    p2[DH:, DH:] = p
    return np.ascontiguousarray(p2.T)


def _build():
    if "nc" in _cache:
        return _cache["nc"]

    import concourse.mybir as mybir
    import concourse.tile as tile
    from concourse import bacc

    F32 = mybir.dt.float32
    BF16 = mybir.dt.bfloat16
    EXP = mybir.ActivationFunctionType.Exp

    nc = bacc.Bacc("TRN2", target_bir_lowering=False, debug=False)
    xT_d = nc.dram_tensor("xT", [DIM, N], BF16, kind="ExternalInput")
    w_d = nc.dram_tensor("wqkv", [DIM, 768], BF16, kind="ExternalInput")
    wout_d = nc.dram_tensor("wout", [G * DH, DIM], BF16, kind="ExternalInput")
    cos_d = nc.dram_tensor("cos2", [128, N], BF16, kind="ExternalInput")
    sin_d = nc.dram_tensor("sin2", [128, N], BF16, kind="ExternalInput")
    p2t_d = nc.dram_tensor("p2t", [128, 128], BF16, kind="ExternalInput")
    part_d = nc.dram_tensor("part", [N, DIM], BF16, kind="ExternalOutput")

    with tile.TileContext(nc) as tc:
        with tc.tile_pool(name="persist", bufs=1) as persist, \
             tc.tile_pool(name="att", bufs=5) as att, \
             tc.tile_pool(name="norm_w", bufs=2) as norm_w, \
             tc.tile_pool(name="outp", bufs=3) as outp, \
             tc.tile_pool(name="xph", bufs=1) as xph, \
             tc.tile_pool(name="rope_w", bufs=2) as rope_w, \
             tc.tile_pool(name="ps", bufs=3, space="PSUM") as ps, \
             tc.tile_pool(name="pso", bufs=2, space="PSUM") as pso:

            # ---- persistent tiles ----
            qk_sb = [persist.tile([128, N], BF16, tag=f"qk{m}", name=f"qk{m}")
                     for m in range(4)]          # q01T, q23T, k01T, k23T
            v_aug = persist.tile([128, NT, G, DH + 1], BF16, tag="vaug")
            wout_sb = [persist.tile([128, DIM], BF16, tag=f"wo{kk}", name=f"wo{kk}")
                       for kk in range(2)]
            outT = [persist.tile([128, N], BF16, tag=f"outT{p}", name=f"outT{p}")
                    for p in range(2)]

            # ---- phase-1 tiles ----
            xT = [xph.tile([128, N], BF16, tag=f"xT{k}", name=f"xT{k}")
                  for k in range(KT)]
            wqkv = [xph.tile([128, 768], BF16, tag=f"wq{k}", name=f"wq{k}")
                    for k in range(KT)]
            cos2 = xph.tile([128, N], BF16, tag="cos2")
            sin2 = xph.tile([128, N], BF16, tag="sin2")
            p2t = xph.tile([128, 128], BF16, tag="p2t")
            ones_col = xph.tile([128, NT, G, 1], F32, tag="ones")

            # ---- input DMA in consumption order ----
            xT_r = xT_d.ap().rearrange("(t p) n -> t p n", p=128)
            w_r = w_d.ap().rearrange("(t p) m -> t p m", p=128)
            for k in range(KT):
                nc.sync.dma_start(out=xT[k][:, 0:1024], in_=xT_r[k][:, 0:1024])
                nc.sync.dma_start(out=wqkv[k], in_=w_r[k])
            nc.sync.dma_start(out=cos2[:, 0:1024], in_=cos_d.ap()[:, 0:1024])
            nc.sync.dma_start(out=sin2[:, 0:1024], in_=sin_d.ap()[:, 0:1024])
            nc.sync.dma_start(out=p2t, in_=p2t_d.ap())
            for k in range(KT):
                nc.sync.dma_start(out=xT[k][:, 1024:2048],
                                  in_=xT_r[k][:, 1024:2048])
            nc.sync.dma_start(out=cos2[:, 1024:2048],
                              in_=cos_d.ap()[:, 1024:2048])
            nc.sync.dma_start(out=sin2[:, 1024:2048],
                              in_=sin_d.ap()[:, 1024:2048])
            for kk in range(2):
                nc.sync.dma_start(
                    out=wout_sb[kk],
                    in_=wout_d.ap().rearrange("(t p) m -> t p m", p=128)[kk])
            nc.vector.memset(ones_col, 1.0)

            def qk_pass0(cpass, ms=(0, 2)):
                """k-outer accumulation of pair-0 q/k tiles for one 1024-wide
                column chunk; consumes each xT k-tile as its DMA lands."""
                csl = slice(cpass * 1024, (cpass + 1) * 1024)
                psq = [ps.tile([128, 1024], F32, tag="s", name=f"qk_ps{m2}")
                       for m2 in range(len(ms))]
                for k in range(KT):
                    for m2, m in enumerate(ms):
                        for half in range(2):
                            hsl = slice(half * 512, (half + 1) * 512)
                            xsl = slice(cpass * 1024 + half * 512,
                                        cpass * 1024 + (half + 1) * 512)
                            nc.tensor.matmul(
                                psq[m2][:, hsl],
                                wqkv[k][:, m * 128:(m + 1) * 128],
                                xT[k][:, xsl],
                                start=(k == 0), stop=(k == KT - 1))
                for m2, m in enumerate(ms):
                    nc.vector.tensor_copy(qk_sb[m][:, csl], psq[m2])

            def rope_chunk(m, c, on_gpsimd=False):
                """RoPE on one 512-col chunk of qk_sb[m]: rotate-half via the
                PE permutation matmul, then combine with cos/sin.  The
                PSUM-reading sin-multiply must stay on DVE; the SBUF-only
                combine can run on the idle GpSimd engine so the phase-1-tail
                rope work does not clog DVE at the attention-window entry."""
                csl = slice(c * 512, (c + 1) * 512)
                rp = ps.tile([128, 1024], F32, tag="s", name="rot")
                rps = rp[:, 0:512]
                nc.tensor.matmul(rps, p2t, qk_sb[m][:, csl],
                                 start=True, stop=True)
                tmp = rope_w.tile([128, 512], BF16, tag="rtmp")
                nc.vector.tensor_mul(tmp, rps, sin2[:, csl])
                eng = nc.gpsimd if on_gpsimd else nc.vector
                eng.tensor_mul(qk_sb[m][:, csl], qk_sb[m][:, csl],
                               cos2[:, csl])
                eng.tensor_add(qk_sb[m][:, csl], qk_sb[m][:, csl], tmp)

            def v_group(tn, cp_act=False):
                mm_ps = ps.tile([128, 1024], F32, tag="s", name="mm_v")
                for k in range(KT):
                    nc.tensor.matmul(
                        mm_ps[:, 0:G * DH],
                        xT[k][:, tn * 128:(tn + 1) * 128],
                        wqkv[k][:, 512:768],
                        start=(k == 0), stop=(k == KT - 1))
                src = mm_ps[:, 0:G * DH].rearrange("p (h d) -> p h d", h=G)
                if cp_act:
                    nc.scalar.copy(v_aug[:, tn, :, 0:DH], src)
                else:
                    nc.vector.tensor_copy(v_aug[:, tn, :, 0:DH], src)
                nc.vector.tensor_copy(v_aug[:, tn, :, DH:DH + 1],
                                      ones_col[:, tn])

            # pair-1 QKV as filler jobs: k-QUARTER per (m2, 512-col chunk) so
            # each job holds a PSUM buf only ~1us; halves are combined via an
            # SBUF staging tile and a DVE add.
            p1_stage = {}

            def pass1_q(m2, chunk, q):
                m = 1 if m2 == 0 else 3
                csl = slice(chunk * 512, (chunk + 1) * 512)
                tt = ps.tile([128, 1024], F32, tag="s", name="p1")
                t = tt[:, 0:512]
                for k in range(q * 4, q * 4 + 4):
                    nc.tensor.matmul(
                        t,
                        wqkv[k][:, m * 128:(m + 1) * 128], xT[k][:, csl],
                        start=(k == q * 4), stop=(k == q * 4 + 3))
                if q == 0:
                    st = rope_w.tile([128, 512], F32, tag="p1st")
                    p1_stage[(m2, chunk)] = st
                    nc.vector.tensor_copy(st, t)
                else:
                    st = p1_stage.pop((m2, chunk))
                    nc.vector.tensor_add(qk_sb[m][:, csl], st, t)

            def normalize(p, iq, o_ps):
                """PV accumulators -> normalized bf16 rows of outT.  The two
                head chains are interleaved stage-by-stage so their DMA and
                gpsimd latencies overlap instead of serializing."""
                isl = slice(iq * 512, iq * 512 + 512)
                o_sb = []
                recips = []
                bcs = []
                for hh in range(2):
                    t = norm_w.tile([DH + 1, 512], F32, tag=f"osb{hh}",
                                    name=f"osb{hh}")
                    nc.vector.tensor_copy(t, o_ps[hh])
                    o_sb.append(t)
                for hh in range(2):
                    r = norm_w.tile([1, 512], F32, tag=f"r0{hh}",
                                    name=f"r0{hh}")
                    nc.sync.dma_start(out=r, in_=o_sb[hh][DH:DH + 1, :])
                    recips.append(r)
                for hh in range(2):
                    nc.vector.reciprocal_approx_fast(recips[hh], recips[hh])
                for hh in range(2):
                    bc = norm_w.tile([DH, 512], F32, tag=f"bc{hh}",
                                     name=f"bc{hh}")
                    nc.gpsimd.partition_broadcast(bc, recips[hh])
                    bcs.append(bc)
                nc.vector.tensor_mul(outT[p][0:DH, isl],
                                     o_sb[0][0:DH, :], bcs[0])
                tmpb = norm_w.tile([DH, 512], BF16, tag="tmpb")
                nc.vector.tensor_mul(tmpb, o_sb[1][0:DH, :], bcs[1])
                nc.sync.dma_start(out=outT[p][DH:2 * DH, isl], in_=tmpb)

            def emit_pv(p, o_ps, jj, exps):
                for hh in range(2):
                    for half in range(2):
                        j = 2 * jj + half
                        nc.tensor.matmul(
                            o_ps[hh],
                            v_aug[:, j, 2 * p + hh, :],
                            exps[hh][:, half * 512:(half + 1) * 512],
                            start=(j == 0), stop=(j == NT - 1))

            def run_attention(blocks):
                """All attention blocks as ONE continuous scores -> exp -> PV
                pipeline.  PV lags exp by one iteration ACROSS block
                boundaries, so each next block's first scores sit between a
                block's last exp and its last PV in PE program order — the
                exp stream never waits out a block tail.  Filler jobs pop
                into PE slack at the given j-slots."""
                pend = None      # (p, iq, o_ps, jj, exps)
                for p, iq, fillers, slots in blocks:
                    o_ps = [pso.tile([DH + 1, 512], F32, tag="o",
                                     name=f"o{hh}") for hh in range(2)]
                    qT = qk_sb[p]
                    kTt = qk_sb[2 + p]
                    isl = slice(iq * 512, iq * 512 + 512)
                    fillers = list(fillers)
                    if slots is None:
                        slots = list(range(len(fillers)))
                    fi = 0
                    for jj in range(NT // 2):
                        s_ps = [ps.tile([128, 1024], F32, tag="s",
                                        name=f"s{hh}") for hh in range(2)]
                        for half in range(2):
                            j = 2 * jj + half
                            jsl = slice(j * 128, (j + 1) * 128)
                            for hh in range(2):
                                hsl = slice(hh * DH, (hh + 1) * DH)
                                # K=64 pair in disjoint PE row groups can
                                # co-issue (explicit tile_position)
                                nc.tensor.matmul(
                                    s_ps[hh][:, half * 512:(half + 1) * 512],
                                    kTt[hsl, jsl], qT[hsl, isl],
                                    start=True, stop=True,
                                    tile_position=(hh * DH, 0))
                        exps = []
                        for hh in range(2):
                            expT = att.tile([128, 1024], BF16, tag="exp")
                            nc.scalar.activation(expT, s_ps[hh], EXP,
                                                 scale=SCALE)
                            exps.append(expT)
                        if pend is not None:
                            pp, piq, po, pjj, pexps = pend
                            emit_pv(pp, po, pjj, pexps)
                            if pjj == NT // 2 - 1:
                                normalize(pp, piq, po)
                        pend = (p, iq, o_ps, jj, exps)
                        while fi < len(fillers) and fi < len(slots) \
                                and slots[fi] <= jj:
                            fillers[fi]()
                            fi += 1
                    while fi < len(fillers):
                        fillers[fi]()
                        fi += 1
                pp, piq, po, pjj, pexps = pend
                emit_pv(pp, po, pjj, pexps)
                normalize(pp, piq, po)

            def proj_tile(tn, cp_act=False):
                nsl = slice(tn * 128, (tn + 1) * 128)
                out_sb = outp.tile([128, DIM], BF16, tag="osb")
                f_ps = ps.tile([128, 1024], F32, tag="s", name="f_ps")
                for c2 in range(2):
                    c2sl = slice(c2 * 512, (c2 + 1) * 512)
                    for kk in range(2):
                        nc.tensor.matmul(
                            f_ps[:, c2sl],
                            outT[kk][:, nsl], wout_sb[kk][:, c2sl],
                            start=(kk == 0), stop=(kk == 1))
                if cp_act:
                    nc.scalar.copy(out_sb, f_ps)
                else:
                    nc.vector.tensor_copy(out_sb, f_ps)
                nc.sync.dma_start(
                    out=part_d.ap().rearrange("(t p) m -> t p m", p=128)[tn],
                    in_=out_sb)

            # ---- emission order ----
            # Inline phase 1 is the minimum needed by attention(0,0):
            # pair-0 qk for columns 0:1024 (k-outer, consumes xT as the DMA
            # lands), its rope, and v tiles 0-7 (copies on the idle scalar
            # engine).  Everything else — pair-0 columns 1024:2048, ALL of
            # pair-1 qkv+rope, v8-15, and the projection — is filler jobs
            # inside the Act-paced attention window.
            qk_pass0(0)
            for m in (0, 2):
                for c in (0, 1):
                    rope_chunk(m, c)
            for tn in range(0, 8):
                v_group(tn, cp_act=True)
            qk_pass0(1)
            for m in (0, 2):
                for c in (2, 3):
                    rope_chunk(m, c, on_gpsimd=True)

            def F(fn, *a):
                return lambda: fn(*a)

            # Deadlines: attention(p, iq) scores at jj=4/6 read k-chunks
            # 2/3 (filler+rope one block earlier or early-slot same block);
            # attention(1,*) reads roped qk_sb[1]/[3] produced in the
            # (0,2)/(0,3) blocks.  proj(iq) lags >= 2 blocks.
            f_v = [F(v_group, tn) for tn in range(8, 16)]
            f_p1 = [F(pass1_q, m2, c, q)
                    for m2 in range(2) for c in range(4) for q in range(2)]
            f_r1 = [F(rope_chunk, m, c) for m in (1, 3) for c in range(4)]

            run_attention([
                # v fillers start at slot 1: a slot-0 filler lands exactly in
                # the score-pipeline fill of the first block (v15 runs as the
                # post-loop leftover, still ahead of the cross-block PV(7))
                (0, 0, f_v, [1, 2, 3, 4, 5, 6, 7]),
                (0, 1, f_p1[0:8], None),
                (0, 2, f_p1[8:16], None),
                (0, 3, f_r1, None),
                (1, 0, [], None),
                (1, 1, [F(proj_tile, tn) for tn in range(0, 4)],
                 [4, 5, 6, 7]),
                (1, 2, [F(proj_tile, tn) for tn in range(4, 8)],
                 [4, 5, 6, 7]),
                (1, 3, [F(proj_tile, tn) for tn in range(8, 10)], [5, 7]),
            ])
            # proj 10,11 depend only on normalize(1,2): they keep the PE warm
            # while normalize(1,3) drains; 12-15 follow it, staged into one
            # SBUF tile and written with a single DMA (one issue+sem instead
            # of four on the critical tail).
            for tn in range(10, 12):
                proj_tile(tn)
            tail_sb = persist.tile([128, 4, DIM], BF16, tag="tail")
            for ti, tn in enumerate(range(12, 16)):
                nsl = slice(tn * 128, (tn + 1) * 128)
                f_ps = ps.tile([128, 1024], F32, tag="s", name="f_ps")
                for c2 in range(2):
                    c2sl = slice(c2 * 512, (c2 + 1) * 512)
                    for kk in range(2):
                        nc.tensor.matmul(
                            f_ps[:, c2sl],
                            outT[kk][:, nsl], wout_sb[kk][:, c2sl],
                            start=(kk == 0), stop=(kk == 1))
                if tn % 2 == 0:
                    nc.scalar.copy(tail_sb[:, ti], f_ps)
                else:
                    nc.vector.tensor_copy(tail_sb[:, ti], f_ps)
            nc.sync.dma_start(
                out=part_d.ap().rearrange("(t p) m -> p t m", p=128)[:, 12:16],
                in_=tail_sb)
    nc.compile()
    _cache["nc"] = nc
    return nc


def kernel(x, w_qkv, w_out, b_out, _trace=False):
    import ml_dtypes
    from concourse.bass_utils import run_bass_kernel_spmd

    x = np.asarray(x, dtype=np.float32)
    w_qkv = np.asarray(w_qkv, dtype=np.float32)
    w_out = np.asarray(w_out, dtype=np.float32)
    b_out = np.asarray(b_out, dtype=np.float32)

    cos2, sin2 = _rope_tables()
    p2t = _p2t()

    in_maps = []
    for c in range(N_CORES):
        b, g = divmod(c, G)
        cols = []
        for blk in range(2):                      # q block, k block
            base = blk * H * DH + g * G * DH
            cols.append(w_qkv[:, base:base + G * DH])
        cols.append(w_qkv[:, 2 * H * DH + g * G * DH:
                          2 * H * DH + (g + 1) * G * DH])   # v block
        wqkv_c = np.ascontiguousarray(np.concatenate(cols, axis=1))  # [DIM,768]
        wout_c = np.ascontiguousarray(
            w_out[g * G * DH:(g + 1) * G * DH, :]).astype(ml_dtypes.bfloat16)
        in_maps.append({
            "xT": np.ascontiguousarray(x[b].T).astype(ml_dtypes.bfloat16),
            "wqkv": wqkv_c.astype(ml_dtypes.bfloat16),
            "wout": wout_c,
            "cos2": cos2.astype(ml_dtypes.bfloat16),
            "sin2": sin2.astype(ml_dtypes.bfloat16),
            "p2t": p2t.astype(ml_dtypes.bfloat16),
        })

    nc = _build()
    res = run_bass_kernel_spmd(nc, in_maps, core_ids=list(range(N_CORES)),
                               trace=_trace)
    out = np.empty((B, N, DIM), dtype=np.float32)
    for b in range(B):
        acc = res.results[G * b]["part"].astype(np.float32)
        for g in range(1, G):
            acc += res.results[G * b + g]["part"].astype(np.float32)
        out[b] = acc + b_out
    if _trace:
        kernel.last_results = res
    return out
